# revision 5
# baseline (speedup 1.0000x reference)
"""ForgetMult linear recurrence h_t = f_t*x_t + (1-f_t)*h_{t-1} on 8 trn2 cores.

Sharding: batch dim B=64 split across 8 cores (8 batches/core, C=8192
independent (b,h) scan channels per core).

The fp32 version of this kernel is pinned at the HBM roofline (96MB/core at
~340GB/s ≈ 290us), so this version moves all device I/O to bf16 (48MB/core):
tensor_tensor_scan keeps its carry in fp32 regardless of operand dtype, so
the recurrence itself doesn't accumulate bf16 rounding — only the per-element
input/output quantization shows up (measured ~3e-3 rel err vs the fp32
reference, gate is 2e-2).

The host pre-packs each core's inputs as bf16 in a partition-major layout
[128, NG*T] where row p holds channel g*128+p for every group g at offset
g*T — so every DMA is [128 partitions x 16KB contiguous rows] (2MB per
descriptor batch, ~line-rate), and no on-device transpose is needed at all
(no PE, no PSUM).

Per core, per chunk of GC=8 groups:
  - sync DMA in: f, x bf16 [128, 8192]
  - ACT: a = 1 - f (bf16 out)
  - DVE: b = f * x (bf16 out, 2 elem/cycle packed mode)
  - DVE: 8x tensor_tensor_scan over [128, 1024] group slices (fp32 carry)
  - scalar DMA out: y bf16 [128, 8192]
Host unpacks y back to [T, B, H] fp32.
"""

import numpy as np
import ml_dtypes

import concourse.bacc as bacc
import concourse.bass as bass
import concourse.mybir as mybir
from concourse import bass_utils
from concourse.tile import TileContext

T = 1024
B = 64
H = 1024
NCORES = 8
BS = B // NCORES  # batches per core
C = BS * H  # channels per core (independent scans)
G = 128  # channels per group == partition dim
NG = C // G  # 64 groups per core
GC = 4  # groups per chunk
W = GC * T  # chunk free width (elements per partition row)
NCHUNK = NG // GC
GC_DVE = GC // 2  # groups of each chunk whose f*x mult runs on DVE (rest GpSimd)

F32 = mybir.dt.float32
BF16 = mybir.dt.bfloat16
BF = ml_dtypes.bfloat16


def build_program() -> bass.Bass:
    nc = bacc.Bacc(trn_type="TRN2")
    f_d = nc.dram_tensor("f", (G, NG * T), BF16, kind="ExternalInput")
    x_d = nc.dram_tensor("x", (G, NG * T), BF16, kind="ExternalInput")
    h0_d = nc.dram_tensor("h0", (G, NG), BF16, kind="ExternalInput")
    y_d = nc.dram_tensor("y", (G, NG * T), BF16, kind="ExternalOutput")

    with TileContext(nc) as tc:
        with (
            tc.tile_pool(name="consts", bufs=1) as consts,
            tc.tile_pool(name="io", bufs=3) as io,
            tc.tile_pool(name="mid", bufs=2) as mid,
            tc.tile_pool(name="outp", bufs=2) as outp,
        ):
            h0t = consts.tile([G, NG], BF16)
            nc.sync.dma_start(out=h0t[:, :], in_=h0_d[:, :])

            for c in range(NCHUNK):
                cs = slice(c * W, (c + 1) * W)
                ft = io.tile([G, W], BF16, tag="f")
                xt = io.tile([G, W], BF16, tag="x")
                nc.sync.dma_start(out=ft[:, :], in_=f_d[:, cs])
                nc.sync.dma_start(out=xt[:, :], in_=x_d[:, cs])
                at = mid.tile([G, W], BF16, tag="a")
                nc.scalar.activation(
                    at[:, :],
                    ft[:, :],
                    mybir.ActivationFunctionType.Copy,
                    bias=1.0,
                    scale=-1.0,
                )
                # b = f*x split across DVE (fast, but scan-bound) and GpSimd
                # (slow, but otherwise idle) so the scans own most of DVE.
                bt = mid.tile([G, W], BF16, tag="b")
                dsl = slice(0, GC_DVE * T)
                gsl = slice(GC_DVE * T, W)
                nc.vector.tensor_tensor(
                    out=bt[:, dsl],
                    in0=ft[:, dsl],
                    in1=xt[:, dsl],
                    op=mybir.AluOpType.mult,
                )
                nc.gpsimd.tensor_tensor(
                    out=bt[:, gsl],
                    in0=ft[:, gsl],
                    in1=xt[:, gsl],
                    op=mybir.AluOpType.mult,
                )
                yt = outp.tile([G, W], BF16, tag="y")
                for i in range(GC):
                    g = c * GC + i
                    sl = slice(i * T, (i + 1) * T)
                    nc.vector.tensor_tensor_scan(
                        out=yt[:, sl],
                        data0=at[:, sl],
                        data1=bt[:, sl],
                        initial=h0t[:, g : g + 1],
                        op0=mybir.AluOpType.mult,
                        op1=mybir.AluOpType.add,
                    )
                nc.scalar.dma_start(out=y_d[:, cs], in_=yt[:, :])
    if not nc.is_finalized():
        nc.finalize()
    return nc


def _pack(a: np.ndarray) -> np.ndarray:
    """[T, B, H] fp32 -> [NCORES, G, NG*T] bf16, partition-major interleave."""
    v = a.astype(BF).reshape(T, NCORES, NG, G)
    return np.ascontiguousarray(v.transpose(1, 3, 2, 0)).reshape(NCORES, G, NG * T)


def run(inputs: dict, trace: bool = False, tmpdir=None) -> tuple[np.ndarray, object]:
    f = np.asarray(inputs["f"], dtype=np.float32)
    x = np.asarray(inputs["x"], dtype=np.float32)
    h0 = np.asarray(inputs["hidden_init"], dtype=np.float32)

    fi = _pack(f)
    xi = _pack(x)
    h0i = np.ascontiguousarray(
        h0.astype(BF).reshape(NCORES, NG, G).transpose(0, 2, 1)
    )  # [NCORES, G, NG]

    nc = build_program()
    in_maps = [
        {"f": fi[m], "x": xi[m], "h0": h0i[m]} for m in range(NCORES)
    ]
    res = bass_utils.run_bass_kernel_spmd(
        nc, in_maps, core_ids=list(range(NCORES)), trace=trace, tmpdir=tmpdir
    )
    # y arrives [G, NG*T] bf16 per core; restore [T, B, H] fp32
    y = np.stack([r["y"].reshape(G, NG, T) for r in res.results])  # [M, G, NG, T]
    out = (
        np.ascontiguousarray(y.transpose(3, 0, 2, 1))
        .reshape(T, B, H)
        .astype(np.float32)
    )
    return out, res


def kernel(**inputs) -> np.ndarray:
    out, _ = run(inputs, trace=False)
    return out


# revision 6
# speedup vs baseline: 1.0189x; 1.0189x over previous
"""ForgetMult linear recurrence h_t = f_t*x_t + (1-f_t)*h_{t-1} on 8 trn2 cores.

Sharding: batch dim B=64 split across 8 cores (8 batches/core, C=8192
independent (b,h) scan channels per core).

The fp32 version of this kernel is pinned at the HBM roofline (96MB/core at
~340GB/s ≈ 290us), so this version moves all device I/O to bf16 (48MB/core):
tensor_tensor_scan keeps its carry in fp32 regardless of operand dtype, so
the recurrence itself doesn't accumulate bf16 rounding — only the per-element
input/output quantization shows up (measured ~3e-3 rel err vs the fp32
reference, gate is 2e-2).

The host pre-packs each core's inputs as bf16 in a partition-major layout
[128, NG*T] where row p holds channel g*128+p for every group g at offset
g*T — so every DMA is [128 partitions x 16KB contiguous rows] (2MB per
descriptor batch, ~line-rate), and no on-device transpose is needed at all
(no PE, no PSUM).

Per core, per chunk of GC=8 groups:
  - sync DMA in: f, x bf16 [128, 8192]
  - ACT: a = 1 - f (bf16 out)
  - DVE: b = f * x (bf16 out, 2 elem/cycle packed mode)
  - DVE: 8x tensor_tensor_scan over [128, 1024] group slices (fp32 carry)
  - scalar DMA out: y bf16 [128, 8192]
Host unpacks y back to [T, B, H] fp32.
"""

import numpy as np
import ml_dtypes

import concourse.bacc as bacc
import concourse.bass as bass
import concourse.mybir as mybir
from concourse import bass_utils
from concourse.tile import TileContext

T = 1024
B = 64
H = 1024
NCORES = 8
BS = B // NCORES  # batches per core
C = BS * H  # channels per core (independent scans)
G = 128  # channels per group == partition dim
NG = C // G  # 64 groups per core
GC = 4  # groups per chunk
W = GC * T  # chunk free width (elements per partition row)
NCHUNK = NG // GC
GC_DVE = GC // 2  # groups of each chunk whose f*x mult runs on DVE (rest GpSimd)

F32 = mybir.dt.float32
BF16 = mybir.dt.bfloat16
BF = ml_dtypes.bfloat16


def build_program() -> bass.Bass:
    nc = bacc.Bacc(trn_type="TRN2")
    f_d = nc.dram_tensor("f", (G, NG * T), BF16, kind="ExternalInput")
    x_d = nc.dram_tensor("x", (G, NG * T), BF16, kind="ExternalInput")
    h0_d = nc.dram_tensor("h0", (G, NG), BF16, kind="ExternalInput")
    y_d = nc.dram_tensor("y", (G, NG * T), BF16, kind="ExternalOutput")

    with TileContext(nc) as tc:
        with (
            tc.tile_pool(name="consts", bufs=1) as consts,
            tc.tile_pool(name="io", bufs=3) as io,
            tc.tile_pool(name="mid", bufs=2) as mid,
            tc.tile_pool(name="outp", bufs=2) as outp,
        ):
            h0t = consts.tile([G, NG], BF16)
            nc.sync.dma_start(out=h0t[:, :], in_=h0_d[:, :])

            for c in range(NCHUNK):
                cs = slice(c * W, (c + 1) * W)
                ft = io.tile([G, W], BF16, tag="f")
                xt = io.tile([G, W], BF16, tag="x")
                nc.sync.dma_start(out=ft[:, :], in_=f_d[:, cs])
                nc.sync.dma_start(out=xt[:, :], in_=x_d[:, cs])
                at = mid.tile([G, W], BF16, tag="a")
                nc.scalar.activation(
                    at[:, :],
                    ft[:, :],
                    mybir.ActivationFunctionType.Copy,
                    bias=1.0,
                    scale=-1.0,
                )
                # b = f*x on DVE only: GpSimd TT measured slower AND its SBUF
                # traffic slowed concurrent DVE scans ~38% (port contention).
                bt = mid.tile([G, W], BF16, tag="b")
                nc.vector.tensor_tensor(
                    out=bt[:, :],
                    in0=ft[:, :],
                    in1=xt[:, :],
                    op=mybir.AluOpType.mult,
                )
                yt = outp.tile([G, W], BF16, tag="y")
                for i in range(GC):
                    g = c * GC + i
                    sl = slice(i * T, (i + 1) * T)
                    nc.vector.tensor_tensor_scan(
                        out=yt[:, sl],
                        data0=at[:, sl],
                        data1=bt[:, sl],
                        initial=h0t[:, g : g + 1],
                        op0=mybir.AluOpType.mult,
                        op1=mybir.AluOpType.add,
                    )
                nc.scalar.dma_start(out=y_d[:, cs], in_=yt[:, :])
    if not nc.is_finalized():
        nc.finalize()
    return nc


def _pack(a: np.ndarray) -> np.ndarray:
    """[T, B, H] fp32 -> [NCORES, G, NG*T] bf16, partition-major interleave."""
    v = a.astype(BF).reshape(T, NCORES, NG, G)
    return np.ascontiguousarray(v.transpose(1, 3, 2, 0)).reshape(NCORES, G, NG * T)


def run(inputs: dict, trace: bool = False, tmpdir=None) -> tuple[np.ndarray, object]:
    f = np.asarray(inputs["f"], dtype=np.float32)
    x = np.asarray(inputs["x"], dtype=np.float32)
    h0 = np.asarray(inputs["hidden_init"], dtype=np.float32)

    fi = _pack(f)
    xi = _pack(x)
    h0i = np.ascontiguousarray(
        h0.astype(BF).reshape(NCORES, NG, G).transpose(0, 2, 1)
    )  # [NCORES, G, NG]

    nc = build_program()
    in_maps = [
        {"f": fi[m], "x": xi[m], "h0": h0i[m]} for m in range(NCORES)
    ]
    res = bass_utils.run_bass_kernel_spmd(
        nc, in_maps, core_ids=list(range(NCORES)), trace=trace, tmpdir=tmpdir
    )
    # y arrives [G, NG*T] bf16 per core; restore [T, B, H] fp32
    y = np.stack([r["y"].reshape(G, NG, T) for r in res.results])  # [M, G, NG, T]
    out = (
        np.ascontiguousarray(y.transpose(3, 0, 2, 1))
        .reshape(T, B, H)
        .astype(np.float32)
    )
    return out, res


def kernel(**inputs) -> np.ndarray:
    out, _ = run(inputs, trace=False)
    return out


# revision 7
# speedup vs baseline: 1.7586x; 1.7259x over previous
"""ForgetMult linear recurrence h_t = f_t*x_t + (1-f_t)*h_{t-1} on 8 trn2 cores.

Sharding: batch dim B=64 split across 8 cores (8 batches/core, C=8192
independent (b,h) scan channels per core).

Device I/O is bf16 (48MB/core vs 96MB fp32; the harness gate is 2e-2 and
measured rel err is ~3.6e-3 because the scan carry stays fp32 end-to-end).

The stock DVE tensor_tensor_scan runs at 2 cycles/element: its carry loops
from the add stage back to the mult stage, so a single chain must insert a
bubble cycle between elements.  This kernel instead registers a custom DVE
uOp program (AFFINE_SCAN2_ANT) that processes TWO channel groups interleaved
along the free dim: even elements belong to group 2p, odd to group 2p+1.
Each element still gets its carry two pipeline slots back — which is exactly
its own chain's previous element — so the engine streams at 1 element/cycle
with zero bubbles: the scans drop from ~150us to ~75us per core.

The recurrence is seeded through the data itself: each pair's segment is
prefixed with two sentinel elements with f=1, x=h0  =>  a=0, b=h0.  The
a=0 kills whatever carry is in flight (the uOp seeds the carry flop with 0
so the very first product can't be NaN), and b injects the initial state.
This also means one instruction chains any number of pair-segments back to
back, re-seeding itself at each boundary.

Host packs each core's tensors as [128, NPAIR*(2T+2)] bf16 (partition p of
pair-segment q holds channels (2q)*128+p and (2q+1)*128+p interleaved), so
every DMA row is ~16KB contiguous and no on-device transpose is needed.
"""

import numpy as np
import ml_dtypes

import concourse.bacc as bacc
import concourse.bass as bass
import concourse.mybir as mybir
from concourse import bass_utils
from concourse import dve_ops as _dve_ops
from concourse.dve_spec import Spec, Src0, Src1
from concourse.dve_uop import (
    ENABLE,
    AluInp,
    AluOp as UAluOp,
    DveOpSpec,
    InpSel,
    OutPath,
    OutSel,
    Trigger,
    UopConfig,
)
from concourse.tile import TileContext

T = 1024
B = 64
H = 1024
NCORES = 8
BS = B // NCORES  # batches per core
C = BS * H  # channels per core (independent scans)
G = 128  # channels per group == partition dim
NG = C // G  # 64 groups per core
NPAIR = NG // 2  # interleaved group pairs per core
SEG = 2 * T + 2  # elements per pair-segment: 2 sentinels + 2T interleaved
PC = 4  # pairs per chunk (== 8 groups, the config that measured best)
W = PC * SEG  # chunk free width per partition row
NCHUNK = NPAIR // PC
FW = NPAIR * SEG  # full free width

F32 = mybir.dt.float32
BF16 = mybir.dt.bfloat16
BF = ml_dtypes.bfloat16

_OP_NAME = "AFFINE_SCAN2_ANT"


def _scan2_uops() -> list[UopConfig]:
    """Two uOps: a 2-cycle non-consuming seed that zeroes the carry flop,
    then a steady state computing out[k] = a[k]*carry + b[k] at 1 elem/cycle,
    where carry = out[k-2] (block1's a-flop, written every cycle, read by
    block0 two cycles later)."""
    uops = []
    for kind in ("seed", "steady"):
        u = UopConfig()
        u.enable_input(InpSel.SRC_0, 1)  # a  -> block0 PREV_DELAY_0
        u.enable_input(InpSel.SRC_1, 2)  # b  -> block0 PREV_DELAY_1
        u.enable_input(InpSel.ZERO, 3)  # 0  -> block0 PREV_DELAY_2
        for k, blk in enumerate(u.datapath_config):
            if k == 0:
                blk.enable_alu(
                    UAluOp.MULTIPLY, AluInp.PREV_DELAY_0, AluInp.NEXT_ALU_OUT_A
                )
            elif k == 1:
                if kind == "seed":
                    blk.enable_alu(
                        UAluOp.BYPASS, AluInp.PREV_DELAY_2, AluInp.PREV_DELAY_2
                    )
                else:
                    blk.enable_alu(UAluOp.ADD, AluInp.PREV_ALU_OUT, AluInp.PREV_DELAY_1)
                blk.alu_out_a_enable = ENABLE  # the carry flop
            else:
                blk.pass_through_alu()
            blk.pass_through_delay(0, 1, 2)
        if kind == "seed":
            u.repeat_count = 2
            u.trigger = (Trigger.COUNT, Trigger.NONE, Trigger.NONE)
            u.next_uop = (1, 0, 0)
        else:
            u.require_inp0 = ENABLE
            u.require_inp1 = ENABLE
            u.trigger = (Trigger.SRC_TENSOR_DONE, Trigger.NONE, Trigger.NONE)
            u.next_uop = (0, 0, 0)
            u.enable_output(OutSel.ALU_OUT, OutPath.WR0_LO)
        uops.append(u)
    return uops


class _HandDveOp(_dve_ops.DveOp):
    """DveOp whose uOp program is hand-built (the Spec DSL cannot express an
    affine recurrence; its single-op scan() has a one-stage feedback only)."""

    def compile(self, ver):
        key = (self.name, ver)
        cached = _dve_ops._COMPILE_CACHE.get(key)
        if cached is None:
            cached = DveOpSpec(
                name=self.name,
                opcode=_dve_ops.get_dve_sub_opcode(self.name),
                uops=_scan2_uops(),
                rd1_en=True,
            )
            cached.validate(ver)
            _dve_ops._COMPILE_CACHE[key] = cached
        return cached


def _register_scan2():
    for op in _dve_ops.OPS:
        if op.name == _OP_NAME:
            return op
    # reference: numpy semantics for CoreSim only (never exercised on the HW
    # path, but kept honest: interleaved affine scan, carry two elements back)
    def _ref(in0, in1, c0, c1, c2):
        a = np.asarray(in0, np.float32)
        b = np.asarray(in1, np.float32)
        y = np.empty_like(a)
        cm2 = np.zeros(a.shape[0], np.float32)
        cm1 = np.zeros(a.shape[0], np.float32)
        for k in range(a.shape[-1]):
            cur = a[..., k] * cm2 + b[..., k]
            y[..., k] = cur
            cm2, cm1 = cm1, cur
        return y

    op = _HandDveOp(
        name=_OP_NAME,
        spec=Spec(body=Src0 * Src1, reference=_ref),  # dummy body; compile() is hand-built
        subdim=False,
        uops_sha={},
    )
    _dve_ops.OPS.append(op)
    _dve_ops._SUB_OPCODE_FOR_NAME[_OP_NAME] = _dve_ops._CUSTOM_DVE_ROW_BASE + (
        len(_dve_ops.OPS) - 1
    )
    _dve_ops.CUSTOM_DVE_SPECS[_OP_NAME] = op.spec
    return op


AFFINE_SCAN2 = _register_scan2()


def build_program() -> bass.Bass:
    nc = bacc.Bacc(trn_type="TRN2")
    f_d = nc.dram_tensor("f", (G, FW), BF16, kind="ExternalInput")
    x_d = nc.dram_tensor("x", (G, FW), BF16, kind="ExternalInput")
    y_d = nc.dram_tensor("y", (G, FW), BF16, kind="ExternalOutput")

    with TileContext(nc) as tc:
        with (
            tc.tile_pool(name="io", bufs=2) as io,
            tc.tile_pool(name="mid", bufs=2) as mid,
            tc.tile_pool(name="outp", bufs=2) as outp,
        ):
            for c in range(NCHUNK):
                cs = slice(c * W, (c + 1) * W)
                ft = io.tile([G, W], BF16, tag="f")
                xt = io.tile([G, W], BF16, tag="x")
                nc.sync.dma_start(out=ft[:, :], in_=f_d[:, cs])
                nc.sync.dma_start(out=xt[:, :], in_=x_d[:, cs])
                at = mid.tile([G, W], BF16, tag="a")
                nc.scalar.activation(
                    at[:, :],
                    ft[:, :],
                    mybir.ActivationFunctionType.Copy,
                    bias=1.0,
                    scale=-1.0,
                )
                bt = mid.tile([G, 1, W], BF16, tag="b")
                nc.vector.tensor_tensor(
                    out=bt[:, 0, :],
                    in0=ft[:, :],
                    in1=xt[:, :],
                    op=mybir.AluOpType.mult,
                )
                yt = outp.tile([G, W], BF16, tag="y")
                nc.vector._custom_dve(
                    AFFINE_SCAN2,
                    out=yt[:, :],
                    in0=at[:, :],
                    in1=bt[:, :, :],  # 3D -> STT struct (2D src1 stream)
                )
                nc.scalar.dma_start(out=y_d[:, cs], in_=yt[:, :])
    if not nc.is_finalized():
        nc.finalize()
    return nc


def _pack(a: np.ndarray, sent: np.ndarray) -> np.ndarray:
    """[T, B, H] fp32 + sentinels [NCORES, NPAIR, 2, G] -> [NCORES, G, FW] bf16."""
    v = a.astype(BF).reshape(T, NCORES, NPAIR, 2, G).transpose(1, 4, 2, 0, 3)
    body = np.ascontiguousarray(v).reshape(NCORES, G, NPAIR, 2 * T)
    s = sent.astype(BF).transpose(0, 3, 1, 2)  # [NCORES, G, NPAIR, 2]
    return np.concatenate([s, body], axis=-1).reshape(NCORES, G, FW)


def run(inputs: dict, trace: bool = False, tmpdir=None) -> tuple[np.ndarray, object]:
    f = np.asarray(inputs["f"], dtype=np.float32)
    x = np.asarray(inputs["x"], dtype=np.float32)
    h0 = np.asarray(inputs["hidden_init"], dtype=np.float32)

    h0v = h0.reshape(NCORES, NPAIR, 2, G)
    fi = _pack(f, np.ones((NCORES, NPAIR, 2, G), np.float32))  # f sentinel=1 -> a=0
    xi = _pack(x, h0v)  # x sentinel=h0 -> b=h0

    nc = build_program()
    in_maps = [{"f": fi[m], "x": xi[m]} for m in range(NCORES)]
    res = bass_utils.run_bass_kernel_spmd(
        nc, in_maps, core_ids=list(range(NCORES)), trace=trace, tmpdir=tmpdir
    )
    y = np.stack([r["y"] for r in res.results]).reshape(NCORES, G, NPAIR, SEG)
    y = y[:, :, :, 2:].reshape(NCORES, G, NPAIR, T, 2)
    out = (
        np.ascontiguousarray(y.transpose(3, 0, 2, 4, 1))
        .reshape(T, B, H)
        .astype(np.float32)
    )
    return out, res


def kernel(**inputs) -> np.ndarray:
    out, _ = run(inputs, trace=False)
    return out
